# revision 18
# baseline (speedup 1.0000x reference)
"""Trainium2 Bass kernel for BatchGATConv (GAT message passing).

Strategy (8 NeuronCores, SPMD, dst-partitioned):
  - Host: in-degree-sort each core's 2500 dst nodes into 20 tiles of 128 so
    every tile has near-uniform degree; chunk k of a tile holds the k-th
    in-edge of each of the tile's 128 nodes (slot-aligned, so dst-local ==
    partition and no one-hot matmuls are needed). Padded slots point at a
    dedicated zero pad row; their attention logit (host-supplied) is -150.
  - Attention logits el/er are folded on the host (el = feat @ (W*attn_l),
    an O(N*Din*H) matvec) and delivered slot-aligned, so gather rows carry
    only the projected features: 512 bf16 = 1024 B per edge, and the
    own-row gather chunk disappears.
  - All edge-softmax weights (exp(leaky(el+er)) in pair-duplicated layout,
    denominators, reciprocals) are computed up front from the host tables,
    overlapping the projection phase on DVE/ACT.
  - Projection (replicated on all cores, bf16): g[pos] = [ft(b0) | ft(b1)];
    two matmuls share one 2KB PSUM bank so PSUM->SBUF casts are [P,512].
  - Aggregation per tile: dma_gather fetches Kt*128 rows in <=1024-index
    calls rotated over 4 SWDGE queues; ft *= ex with a packed bf16
    innermost pair dim (DVE 2x mode); PSUM segment-sum via identity
    matmuls; out = Prelu(acc * rcp) in f16, cast to f32 on the host.
"""

import numpy as np

try:
    import concourse.bass as bass
except ImportError:  # pragma: no cover
    import sys

    sys.path.insert(0, "/opt/trn_rl_repo")
    import concourse.bass as bass

import ml_dtypes
import concourse.bacc as bacc
import concourse.mybir as mybir
import concourse.tile as tile
from concourse.bass_utils import run_bass_kernel_spmd

P = 128
F32 = mybir.dt.float32
F16 = mybir.dt.float16
BF16 = mybir.dt.bfloat16
I32 = mybir.dt.int32
I16 = mybir.dt.int16

# problem constants
N, B, DIN, H, D, E = 20000, 2, 128, 4, 64, 320000
NEG = 0.2
NCORES = 8
NPC = N // NCORES  # 2500 dst nodes per core
NT = 20  # node tiles per core
NPOSC = NT * P  # 2560 positions per core (60 pad positions)
NPOS = NCORES * NPOSC  # 20480 global positions
PADROW = NPOS  # pad row index (zeros)
GROWS = NPOS + 2
HD = H * D  # 256
BH = B * H  # 8
FT = B * HD  # 512
RW = 512  # g row width: [ft b0 | ft b1] bf16 = 1024 B (%256 for dma_gather)
PAD_EL = -150.0
QBLK = 2048  # projection rows per block
NBLK = NPOS // QBLK  # 10 per batch
NSWQ = 4  # SWDGE queues (Q7 descriptor-gen parallelism)


def _host_prep(src, dst):
    """Degree-sort nodes per core; build per-core wrapped int16 gather tables.

    Returns (K, idxs_all, sl_all, pos2node):
      K: per-tile chunk count, shared across cores (max in-degree in tile)
      idxs_all[c]: [128, sum(K)*8] int16 wrapped gather indices
      sl_all[c]: [sum(K)*P] int64 raw slot->position (PADROW = pad)
      pos2node: [NPOS] original node id per position (-1 = pad)
    """
    deg = np.bincount(dst, minlength=N).astype(np.int64)
    pos2node = np.full(NPOS, -1, np.int64)
    for c in range(NCORES):
        ids = np.arange(c * NPC, (c + 1) * NPC)
        order = np.argsort(-deg[ids], kind="stable")
        pos2node[c * NPOSC : c * NPOSC + NPC] = ids[order]
    node2pos = np.empty(N, np.int64)
    real = pos2node >= 0
    node2pos[pos2node[real]] = np.nonzero(real)[0]

    first = np.arange(NCORES)[:, None] * NPOSC + np.arange(NT)[None, :] * P
    K = np.maximum(deg[pos2node[first]].max(axis=0), 1)  # [NT]

    dpos = node2pos[dst]
    spos = node2pos[src]
    order_e = np.argsort(dpos, kind="stable")
    ds = dpos[order_e]
    ss = spos[order_e]
    starts = np.searchsorted(ds, np.arange(NPOS + 1))
    k_e = np.arange(E) - starts[ds]

    idxs_all = []
    sl_all = []
    for c in range(NCORES):
        cols = []
        slc = []
        for t in range(NT):
            Kt = int(K[t])
            base = c * NPOSC + t * P
            sl = np.full(Kt * P, PADROW, np.int64)
            e0, e1 = starts[base], starts[base + P]
            pp_ = ds[e0:e1] - base
            sl[k_e[e0:e1] * P + pp_] = ss[e0:e1]
            cols.append(sl.reshape(-1, 16).T)
            slc.append(sl)
        idxs_all.append(
            np.ascontiguousarray(
                np.tile(np.concatenate(cols, axis=1), (8, 1))
            ).astype(np.int16)
        )
        sl_all.append(np.concatenate(slc))
    return list(map(int, K)), idxs_all, sl_all, pos2node


def _build(K, stage='all'):
    S8 = sum(k * 8 for k in K)
    KM = max(K)
    TCH = 5  # tiles per softmax chunk
    KC = [max(K[c0 : c0 + TCH]) for c0 in range(0, NT, TCH)]
    SE = sum(TCH * kc * BH for kc in KC)  # per-chunk-padded el table
    AF = mybir.ActivationFunctionType
    OP = mybir.AluOpType

    nc = bacc.Bacc(trn_type="TRN2", num_swdge_queues=NSWQ)
    featT = nc.dram_tensor("featT", [DIN, B * NPOS], BF16, kind="ExternalInput")
    wmat = nc.dram_tensor("wmat", [DIN, HD], BF16, kind="ExternalInput")
    idxsd = nc.dram_tensor("idxs", [128, S8], I16, kind="ExternalInput")
    elsd = nc.dram_tensor("els", [128, SE], BF16, kind="ExternalInput")
    erod = nc.dram_tensor("ero", [128, NT * BH], BF16, kind="ExternalInput")
    outd = nc.dram_tensor("out", [NPOSC, FT], F16, kind="ExternalOutput")
    g = nc.dram_tensor("gtab", [GROWS, RW], BF16)

    with tile.TileContext(nc) as tc:
        with (
            tc.tile_pool(name="const", bufs=1) as cp,
            tc.tile_pool(name="proj", bufs=3) as pp,
            tc.tile_pool(name="projps", bufs=4, space="PSUM") as ppp,
            tc.tile_pool(name="agg", bufs=10) as ag,
            tc.tile_pool(name="small", bufs=3) as sm,
            tc.tile_pool(name="accps", bufs=3, space="PSUM") as psp,
        ):
            # resident constants
            w_sb = cp.tile([DIN, HD], BF16)
            nc.sync.dma_start(w_sb[:], wmat[:])
            idxs_res = cp.tile([128, S8], I16)
            nc.sync.dma_start(idxs_res[:], idxsd[:])
            el_sb = cp.tile([128, SE], BF16)
            nc.sync.dma_start(el_sb[:], elsd[:])
            er_sb = cp.tile([128, NT * BH], BF16)
            nc.sync.dma_start(er_sb[:], erod[:])
            iota_r = cp.tile([P, P], I32)
            nc.gpsimd.iota(iota_r[:], pattern=[[1, P]], base=0, channel_multiplier=0)
            iota_rf = cp.tile([P, P], F32)
            nc.vector.tensor_copy(iota_rf[:], iota_r[:])
            iota_p = cp.tile([P, 1], I32)
            nc.gpsimd.iota(iota_p[:], pattern=[[1, 1]], base=0, channel_multiplier=1)
            iota_pf = cp.tile([P, 1], F32)
            nc.vector.tensor_copy(iota_pf[:], iota_p[:])
            ident = cp.tile([P, P], BF16)
            nc.vector.tensor_scalar(
                out=ident[:],
                in0=iota_rf[:],
                scalar1=iota_pf[:, 0:1],
                scalar2=None,
                op0=OP.is_equal,
            )
            # pad row: ft = 0 (its logit lives in the host-built el table)
            padt = cp.tile([1, RW], BF16)
            nc.gpsimd.memset(padt[:], 0.0)
            nc.sync.dma_start(out=g[NPOS : NPOS + 1, :], in_=padt[:])

            # ---- softmax weights in 4 tile-chunks (few big ops; the
            # pad K-slots carry el=-150 so their exp is ~0 and harmless) ----
            # pair-duplicated er (tiny) so chunk ops stay within TENSOR3D
            er2 = cp.tile([P, NT * BH, 2], BF16)
            nc.vector.tensor_tensor(
                out=er2[:],
                in0=er_sb[:, :, None].to_broadcast([P, NT * BH, 2]),
                in1=er_sb[:, :, None].to_broadcast([P, NT * BH, 2]),
                op=OP.bypass,
            )
            ex_tiles = []
            rcp_tiles = []
            eloff = 0
            for ci, t0 in enumerate(
                range(0, NT if stage in ('all', 'agg') else 0, TCH)
            ):
                Kc = KC[ci]
                ext = cp.tile([P, TCH * Kc * BH * 2], BF16)
                exv = ext[:].rearrange("p (t k q w) -> p t k q w", k=Kc, q=BH, w=2)
                nc.vector.tensor_tensor(
                    out=ext[:].rearrange("p (tkq w) -> p tkq w", w=2),
                    in0=el_sb[:, eloff : eloff + TCH * Kc * BH][
                        :, :, None
                    ].to_broadcast([P, TCH * Kc * BH, 2]),
                    in1=er2[:, t0 * BH : (t0 + TCH) * BH, :]
                    .rearrange("p (t q) w -> p t (q w)", q=BH)[:, :, None, :]
                    .to_broadcast([P, TCH, Kc, 2 * BH]),
                    op=OP.add,
                )
                eloff += TCH * Kc * BH
                nc.scalar.activation(ext[:], ext[:], AF.Prelu, alpha=NEG)
                nc.scalar.activation(ext[:], ext[:], AF.Exp)
                dsum = sm.tile([P, TCH * 2 * BH], F32, tag="dsum")
                nc.vector.tensor_reduce(
                    out=dsum[:].rearrange("p (t qw) -> p t qw", qw=2 * BH),
                    in_=exv.rearrange("p t k q w -> p t (q w) k"),
                    axis=mybir.AxisListType.X,
                    op=OP.add,
                )
                rct = cp.tile([P, TCH * 2 * BH], F32)
                nc.vector.reciprocal(rct[:], dsum[:])
                for tt in range(TCH):
                    ex_tiles.append(
                        ext[:, tt * Kc * BH * 2 : (tt + 1) * Kc * BH * 2]
                    )
                    rcp_tiles.append(rct[:, tt * 2 * BH : (tt + 1) * 2 * BH])

            # ---- projection: g[q] = [ft(b0) | ft(b1)] ----
            hw = (nc.sync, nc.scalar)
            for b in range(B if stage in ('all', 'proj') else 0):
                for blk in range(NBLK):
                    j = b * NBLK + blk
                    ftb = pp.tile([DIN, QBLK], BF16, tag="ftb")
                    hw[j % 2].dma_start(
                        ftb[:],
                        featT[:, b * NPOS + blk * QBLK : b * NPOS + (blk + 1) * QBLK],
                    )
                    pw = pp.tile([P, QBLK // P, HD], BF16, tag="pw")
                    for i in range(0, QBLK // P, 2):
                        po = ppp.tile([P, 2, HD], F32, tag="po")
                        for u in range(2):
                            nc.tensor.matmul(
                                po[:, u, :],
                                lhsT=ftb[:, (i + u) * P : (i + u + 1) * P],
                                rhs=w_sb[:],
                                start=True,
                                stop=True,
                            )
                        if i % 6 == 0:
                            nc.vector.tensor_copy(pw[:, i : i + 2, :], po[:])
                        else:
                            nc.scalar.activation(pw[:, i : i + 2, :], po[:], AF.Copy)
                    dst_ap = g[
                        blk * QBLK : (blk + 1) * QBLK, b * HD : (b + 1) * HD
                    ].rearrange("(i p) c -> p i c", p=P)
                    hw[(j + 1) % 2].dma_start(out=dst_ap, in_=pw[:])

            # ---- aggregation: per tile, slot-aligned weighted segment sum ----
            off8 = 0
            eloff = 0
            ncall = 0
            pending = []

            def _finalize(t, acc, Kt):
                o1 = sm.tile([P, FT], F32, tag="o1")
                nc.vector.tensor_tensor(
                    out=o1[:].rearrange("p (q d2 w) -> p q d2 w", q=BH, w=2),
                    in0=acc[:].rearrange("p (q d2 w) -> p q d2 w", q=BH, w=2),
                    in1=rcp_tiles[t]
                    .rearrange("p (q w) -> p q w", w=2)[:, :, None, :]
                    .to_broadcast([P, BH, D // 2, 2]),
                    op=OP.mult,
                )
                og = sm.tile([P, FT], F16, tag="og")
                nc.scalar.activation(og[:], o1[:], AF.Prelu, alpha=NEG)
                nc.sync.dma_start(out=outd[t * P : (t + 1) * P, :], in_=og[:])

            GG = 8  # chunks per gather group (== SWDGE ring cap of 1024 idx)
            for t in range(NT if stage in ('all', 'agg') else 0):
                Kt = K[t]
                acc = psp.tile([P, FT], F32, tag="acc")
                exv = ex_tiles[t].rearrange("p (k q w) -> p k q w", q=BH, w=2)
                # stream the tile's chunks through small per-call buffers:
                # deep pool (bufs) keeps the Q7 descriptor gen far ahead
                for g0 in range(0, Kt, GG):
                    gn = min(GG, Kt - g0)
                    gt = ag.tile([P, GG, RW], BF16, tag="gt")
                    nc.gpsimd.dma_gather(
                        out_ap=gt[:, 0:gn, :],
                        in_ap=g[:],
                        idxs_ap=idxs_res[:, off8 + g0 * 8 : off8 + (g0 + gn) * 8],
                        num_idxs=gn * P,
                        num_idxs_reg=gn * P,
                        elem_size=RW,
                        queue_num=ncall % NSWQ,
                    )
                    ncall += 1
                    # messages: ft *= ex (in-place; packed bf16 pairs)
                    nc.vector.tensor_tensor(
                        out=gt[:, 0:gn, :].rearrange(
                            "p k (q d2 w) -> p k q d2 w", q=BH, w=2
                        ),
                        in0=gt[:, 0:gn, :].rearrange(
                            "p k (q d2 w) -> p k q d2 w", q=BH, w=2
                        ),
                        in1=exv[:, g0 : g0 + gn, :, None, :].to_broadcast(
                            [P, gn, BH, D // 2, 2]
                        ),
                        op=OP.mult,
                    )
                    # segment sum via identity matmuls accumulating in PSUM
                    for k in range(gn):
                        nc.tensor.matmul(
                            acc[:],
                            lhsT=ident[:],
                            rhs=gt[:, k, :],
                            start=(g0 + k == 0),
                            stop=(g0 + k == Kt - 1),
                        )
                off8 += Kt * 8

                pending.append((t, acc, Kt))
                if len(pending) > 2:
                    _finalize(*pending.pop(0))
            for args in pending:
                _finalize(*args)

    nc.compile()
    return nc


def _make_inputs(feat, W, attn_l, attn_r, src, dst, n_nodes=N, n_cores=NCORES):
    feat = np.asarray(feat, dtype=np.float32)
    W = np.asarray(W, dtype=np.float32)
    attn_l = np.asarray(attn_l, dtype=np.float32)
    attn_r = np.asarray(attn_r, dtype=np.float32)
    src = np.asarray(src)
    dst = np.asarray(dst)

    K, idxs_all, sl_all, pos2node = _host_prep(src, dst)

    real = pos2node >= 0
    ftp = np.zeros((B, NPOS, DIN), np.float32)
    ftp[:, real, :] = feat[pos2node[real]].transpose(1, 0, 2)
    featT = np.ascontiguousarray(ftp.reshape(B * NPOS, DIN).T).astype(
        ml_dtypes.bfloat16
    )
    wmat = W.reshape(DIN, HD).astype(ml_dtypes.bfloat16)

    # host-folded attention logits (analogous to attn folding into W):
    # el/er per position, (NPOS+1, B*H) with the pad row's el = PAD_EL
    Wl = (W.reshape(DIN, H, D) * attn_l[None]).sum(-1)  # [DIN, H]
    Wr = (W.reshape(DIN, H, D) * attn_r[None]).sum(-1)
    el_pos = np.full((NPOS + 1, B, H), PAD_EL, np.float32)
    er_pos = np.zeros((NPOS + 1, B, H), np.float32)
    el_pos[:NPOS][real] = feat[pos2node[real]] @ Wl
    er_pos[:NPOS][real] = feat[pos2node[real]] @ Wr
    el_pos = el_pos.reshape(NPOS + 1, BH)
    er_pos = er_pos.reshape(NPOS + 1, BH)
    TCH = 5  # tiles per softmax chunk (matches _build)
    KC = [max(K[c0 : c0 + TCH]) for c0 in range(0, NT, TCH)]

    in_maps = []
    for c in range(n_cores):
        # slot-aligned el table padded per-chunk to Kc: [128, sum(5*Kc*BH)]
        parts = []
        off = 0
        for t in range(NT):
            Kt = K[t]
            Kc = KC[t // TCH]
            sl = sl_all[c][off : off + Kt * P]
            off += Kt * P
            ev = np.full((Kc, P, BH), PAD_EL, np.float32)
            ev[:Kt] = el_pos[sl].reshape(Kt, P, BH)
            parts.append(ev.transpose(1, 0, 2).reshape(P, Kc * BH))
        els = np.ascontiguousarray(np.concatenate(parts, axis=1)).astype(
            ml_dtypes.bfloat16
        )
        # own-position er table: [128, NT*BH]
        ero = np.ascontiguousarray(
            er_pos[c * NPOSC : (c + 1) * NPOSC]
            .reshape(NT, P, BH)
            .transpose(1, 0, 2)
            .reshape(P, NT * BH)
        ).astype(ml_dtypes.bfloat16)
        in_maps.append(
            {
                "featT": featT,
                "wmat": wmat,
                "idxs": idxs_all[c],
                "els": els,
                "ero": ero,
            }
        )
    return K, in_maps, pos2node


_CACHE = {}


def kernel(feat, W, attn_l, attn_r, src, dst):
    K, in_maps, pos2node = _make_inputs(feat, W, attn_l, attn_r, src, dst)
    key = tuple(K)
    if key not in _CACHE:
        _CACHE[key] = _build(K)
    nc = _CACHE[key]
    res = run_bass_kernel_spmd(nc, in_maps, list(range(NCORES))).results
    out = np.empty((N, B, H, D), np.float32)
    for c in range(NCORES):
        nodes = pos2node[c * NPOSC : c * NPOSC + NPC]
        out[nodes] = res[c]["out"][:NPC].astype(np.float32).reshape(NPC, B, H, D)
    return out


if __name__ == "__main__":
    rng = np.random.default_rng(0)
    feat = rng.standard_normal((N, B, DIN), dtype=np.float32)
    W = rng.standard_normal((DIN, H * D), dtype=np.float32) / np.sqrt(DIN)
    al = rng.standard_normal((H, D), dtype=np.float32) * 0.1
    ar = rng.standard_normal((H, D), dtype=np.float32) * 0.1
    src = rng.integers(0, N, E).astype(np.int32)
    dst = rng.integers(0, N, E).astype(np.int32)
    out = kernel(feat=feat, W=W, attn_l=al, attn_r=ar, src=src, dst=dst)
    print(out.shape, out.dtype, np.abs(out).mean())


# revision 22
# speedup vs baseline: 1.0482x; 1.0482x over previous
"""Trainium2 Bass kernel for BatchGATConv (GAT message passing).

Strategy (8 NeuronCores, SPMD, dst-partitioned):
  - Host: in-degree-sort each core's 2500 dst nodes into 20 tiles of 128 so
    every tile has near-uniform degree; chunk k of a tile holds the k-th
    in-edge of each of the tile's 128 nodes (slot-aligned, so dst-local ==
    partition and no one-hot matmuls are needed). Padded slots point at a
    dedicated zero pad row; their attention logit (host-supplied) is -150.
  - Attention logits el/er are folded on the host (el = feat @ (W*attn_l),
    an O(N*Din*H) matvec) and delivered slot-aligned, so gather rows carry
    only the projected features: 512 bf16 = 1024 B per edge, and the
    own-row gather chunk disappears.
  - All edge-softmax weights (exp(leaky(el+er)) in pair-duplicated layout,
    denominators, reciprocals) are computed up front from the host tables,
    overlapping the projection phase on DVE/ACT.
  - Projection (replicated on all cores, bf16): g[pos] = [ft(b0) | ft(b1)];
    two matmuls share one 2KB PSUM bank so PSUM->SBUF casts are [P,512].
  - Aggregation per tile: dma_gather fetches Kt*128 rows in <=1024-index
    calls rotated over 4 SWDGE queues; ft *= ex with a packed bf16
    innermost pair dim (DVE 2x mode); PSUM segment-sum via identity
    matmuls; out = Prelu(acc * rcp) in f16, cast to f32 on the host.
"""

import numpy as np

try:
    import concourse.bass as bass
except ImportError:  # pragma: no cover
    import sys

    sys.path.insert(0, "/opt/trn_rl_repo")
    import concourse.bass as bass

import ml_dtypes
import concourse.bacc as bacc
import concourse.mybir as mybir
import concourse.tile as tile
from concourse.bass_utils import run_bass_kernel_spmd

P = 128
F32 = mybir.dt.float32
F16 = mybir.dt.float16
BF16 = mybir.dt.bfloat16
I32 = mybir.dt.int32
I16 = mybir.dt.int16

# problem constants
N, B, DIN, H, D, E = 20000, 2, 128, 4, 64, 320000
NEG = 0.2
NCORES = 8
NPC = N // NCORES  # 2500 dst nodes per core
NT = 20  # node tiles per core
NPOSC = NT * P  # 2560 positions per core (60 pad positions)
NPOS = NCORES * NPOSC  # 20480 global positions
PADROW = NPOS  # pad row index (zeros)
GROWS = NPOS + 2
HD = H * D  # 256
BH = B * H  # 8
FT = B * HD  # 512
RW = 512  # g row width: [ft b0 | ft b1] bf16 = 1024 B (%256 for dma_gather)
PAD_EL = -150.0
QBLK = 2048  # projection rows per block
NBLK = NPOS // QBLK  # 10 per batch
NSWQ = 4  # SWDGE queues (Q7 descriptor-gen parallelism)


def _host_prep(src, dst):
    """Degree-sort nodes per core; build per-core wrapped int16 gather tables.

    Returns (K, idxs_all, sl_all, pos2node):
      K: per-tile chunk count, shared across cores (max in-degree in tile)
      idxs_all[c]: [128, sum(K)*8] int16 wrapped gather indices
      sl_all[c]: [sum(K)*P] int64 raw slot->position (PADROW = pad)
      pos2node: [NPOS] original node id per position (-1 = pad)
    """
    deg = np.bincount(dst, minlength=N).astype(np.int64)
    pos2node = np.full(NPOS, -1, np.int64)
    for c in range(NCORES):
        ids = np.arange(c * NPC, (c + 1) * NPC)
        order = np.argsort(-deg[ids], kind="stable")
        pos2node[c * NPOSC : c * NPOSC + NPC] = ids[order]
    node2pos = np.empty(N, np.int64)
    real = pos2node >= 0
    node2pos[pos2node[real]] = np.nonzero(real)[0]

    first = np.arange(NCORES)[:, None] * NPOSC + np.arange(NT)[None, :] * P
    K = np.maximum(deg[pos2node[first]].max(axis=0), 1)  # [NT]

    dpos = node2pos[dst]
    spos = node2pos[src]
    order_e = np.argsort(dpos, kind="stable")
    ds = dpos[order_e]
    ss = spos[order_e]
    starts = np.searchsorted(ds, np.arange(NPOS + 1))
    k_e = np.arange(E) - starts[ds]

    idxs_all = []
    sl_all = []
    for c in range(NCORES):
        cols = []
        slc = []
        for t in range(NT):
            Kt = int(K[t])
            base = c * NPOSC + t * P
            sl = np.full(Kt * P, PADROW, np.int64)
            e0, e1 = starts[base], starts[base + P]
            pp_ = ds[e0:e1] - base
            sl[k_e[e0:e1] * P + pp_] = ss[e0:e1]
            cols.append(sl.reshape(-1, 16).T)
            slc.append(sl)
        idxs_all.append(
            np.ascontiguousarray(
                np.tile(np.concatenate(cols, axis=1), (8, 1))
            ).astype(np.int16)
        )
        sl_all.append(np.concatenate(slc))
    return list(map(int, K)), idxs_all, sl_all, pos2node


def _build(K, stage='all'):
    S8 = sum(k * 8 for k in K)
    KM = max(K)
    TCH = 5  # tiles per softmax chunk
    KC = [max(K[c0 : c0 + TCH]) for c0 in range(0, NT, TCH)]
    SE = sum(TCH * kc * BH for kc in KC)  # per-chunk-padded el table
    AF = mybir.ActivationFunctionType
    OP = mybir.AluOpType

    nc = bacc.Bacc(trn_type="TRN2", num_swdge_queues=NSWQ)
    featT = nc.dram_tensor("featT", [DIN, B * NPOS], BF16, kind="ExternalInput")
    wmat = nc.dram_tensor("wmat", [DIN, HD], BF16, kind="ExternalInput")
    idxsd = nc.dram_tensor("idxs", [128, S8], I16, kind="ExternalInput")
    elsd = nc.dram_tensor("els", [128, SE], BF16, kind="ExternalInput")
    erod = nc.dram_tensor("ero", [128, NT * BH], BF16, kind="ExternalInput")
    outd = nc.dram_tensor("out", [NPOSC, FT], F16, kind="ExternalOutput")
    g = nc.dram_tensor("gtab", [GROWS, RW], BF16)

    with tile.TileContext(nc) as tc:
        with (
            tc.tile_pool(name="const", bufs=1) as cp,
            tc.tile_pool(name="proj", bufs=3) as pp,
            tc.tile_pool(name="projps", bufs=4, space="PSUM") as ppp,
            tc.tile_pool(name="agg", bufs=9) as ag,
            tc.tile_pool(name="small", bufs=3) as sm,
            tc.tile_pool(name="accps", bufs=3, space="PSUM") as psp,
        ):
            # resident constants
            w_sb = cp.tile([DIN, HD], BF16)
            nc.sync.dma_start(w_sb[:], wmat[:])
            idxs_res = cp.tile([128, S8], I16)
            nc.sync.dma_start(idxs_res[:], idxsd[:])
            el_sb = cp.tile([128, SE], BF16)
            nc.sync.dma_start(el_sb[:], elsd[:])
            er_sb = cp.tile([128, NT * BH], BF16)
            nc.sync.dma_start(er_sb[:], erod[:])
            iota_r = cp.tile([P, P], I32)
            nc.gpsimd.iota(iota_r[:], pattern=[[1, P]], base=0, channel_multiplier=0)
            iota_rf = cp.tile([P, P], F32)
            nc.vector.tensor_copy(iota_rf[:], iota_r[:])
            iota_p = cp.tile([P, 1], I32)
            nc.gpsimd.iota(iota_p[:], pattern=[[1, 1]], base=0, channel_multiplier=1)
            iota_pf = cp.tile([P, 1], F32)
            nc.vector.tensor_copy(iota_pf[:], iota_p[:])
            ident = cp.tile([P, P], BF16)
            nc.vector.tensor_scalar(
                out=ident[:],
                in0=iota_rf[:],
                scalar1=iota_pf[:, 0:1],
                scalar2=None,
                op0=OP.is_equal,
            )
            # pad row: ft = 0 (its logit lives in the host-built el table)
            padt = cp.tile([1, RW], BF16)
            nc.gpsimd.memset(padt[:], 0.0)
            nc.sync.dma_start(out=g[NPOS : NPOS + 1, :], in_=padt[:])

            # ---- per-tile softmax weights ----
            ex_tiles = []
            rcp_tiles = []
            eloff = 0
            for t in range(NT if stage in ('all', 'agg') else 0):
                Kt = K[t]
                Kc = KC[t // TCH]
                ext = cp.tile([P, Kc * BH * 2], BF16)
                exv = ext[:].rearrange("p (k q w) -> p k q w", q=BH, w=2)
                nc.vector.tensor_tensor(
                    out=exv[:, 0:Kc, :, :],
                    in0=el_sb[:, eloff : eloff + Kc * BH]
                    .rearrange("p (k q) -> p k q", q=BH)[:, :, :, None]
                    .to_broadcast([P, Kc, BH, 2]),
                    in1=er_sb[:, t * BH : (t + 1) * BH][:, None, :, None]
                    .to_broadcast([P, Kc, BH, 2]),
                    op=OP.add,
                )
                nc.scalar.activation(ext[:], ext[:], AF.Prelu, alpha=NEG)
                nc.scalar.activation(ext[:], ext[:], AF.Exp)
                dsum = sm.tile([P, 2 * BH], F32, tag="dsum")
                nc.vector.tensor_reduce(
                    out=dsum[:],
                    in_=exv.rearrange("p k q w -> p (q w) k"),
                    axis=mybir.AxisListType.X,
                    op=OP.add,
                )
                rct = cp.tile([P, 2 * BH], F32)
                nc.vector.reciprocal(rct[:], dsum[:])
                ex_tiles.append(ext[:])
                rcp_tiles.append(rct[:])
                eloff += Kc * BH
            # ---- projection: g[q] = [ft(b0) | ft(b1)] ----
            hw = (nc.sync, nc.scalar)
            for b in range(B if stage in ('all', 'proj') else 0):
                for blk in range(NBLK):
                    j = b * NBLK + blk
                    ftb = pp.tile([DIN, QBLK], BF16, tag="ftb")
                    hw[j % 2].dma_start(
                        ftb[:],
                        featT[:, b * NPOS + blk * QBLK : b * NPOS + (blk + 1) * QBLK],
                    )
                    pw = pp.tile([P, QBLK // P, HD], BF16, tag="pw")
                    for i in range(0, QBLK // P, 2):
                        po = ppp.tile([P, 2, HD], F32, tag="po")
                        for u in range(2):
                            nc.tensor.matmul(
                                po[:, u, :],
                                lhsT=ftb[:, (i + u) * P : (i + u + 1) * P],
                                rhs=w_sb[:],
                                start=True,
                                stop=True,
                            )
                        if i % 6 == 0:
                            nc.vector.tensor_copy(pw[:, i : i + 2, :], po[:])
                        else:
                            nc.scalar.activation(pw[:, i : i + 2, :], po[:], AF.Copy)
                    dst_ap = g[
                        blk * QBLK : (blk + 1) * QBLK, b * HD : (b + 1) * HD
                    ].rearrange("(i p) c -> p i c", p=P)
                    hw[(j + 1) % 2].dma_start(out=dst_ap, in_=pw[:])

            # ---- aggregation: per tile, slot-aligned weighted segment sum ----
            off8 = 0
            eloff = 0
            ncall = 0
            pending = []

            def _finalize(t, acc, Kt):
                o1 = sm.tile([P, FT], F32, tag="o1")
                nc.vector.tensor_tensor(
                    out=o1[:].rearrange("p (q d2 w) -> p q d2 w", q=BH, w=2),
                    in0=acc[:].rearrange("p (q d2 w) -> p q d2 w", q=BH, w=2),
                    in1=rcp_tiles[t]
                    .rearrange("p (q w) -> p q w", w=2)[:, :, None, :]
                    .to_broadcast([P, BH, D // 2, 2]),
                    op=OP.mult,
                )
                og = sm.tile([P, FT], F16, tag="og")
                nc.scalar.activation(og[:], o1[:], AF.Prelu, alpha=NEG)
                nc.sync.dma_start(out=outd[t * P : (t + 1) * P, :], in_=og[:])

            GG = 8  # chunks per gather group (== SWDGE ring cap of 1024 idx)
            # prologue: descriptor-gen the first calls (7 chunks each, one
            # per queue) with prepare_only DURING the projection; the
            # triggers inherit the g-write dependency and fire the DMAs the
            # moment g is ready (Tile-managed count=None path).
            PRO = 7  # chunks per prologue call (fits an untriggered ring)
            NPRO = (
                min(NSWQ, K[0] // PRO) if stage in ('all', 'agg') else 0
            )
            dma_sems = [
                nc.alloc_semaphore(f"swdge_prep_dma{q}") for q in range(NPRO)
            ]
            pro_tiles = []
            for j in range(NPRO):
                gtp = cp.tile([P, PRO, RW], BF16)
                nc.gpsimd.dma_gather(
                    out_ap=gtp[:],
                    in_ap=g[:],
                    idxs_ap=idxs_res[:, j * PRO * 8 : (j + 1) * PRO * 8],
                    num_idxs=PRO * P,
                    num_idxs_reg=PRO * P,
                    elem_size=RW,
                    queue_num=j % NSWQ,
                    prepare_only=True,
                    sem=dma_sems[j],
                )
                pro_tiles.append(gtp)
            for j in range(NPRO):
                nc.gpsimd.trigger_dma(count=None, queue_num=j % NSWQ)
            ncall = NPRO

            for t in range(NT if stage in ('all', 'agg') else 0):
                Kt = K[t]
                acc = psp.tile([P, FT], F32, tag="acc")
                exv = ex_tiles[t].rearrange("p (k q w) -> p k q w", q=BH, w=2)
                # group list: tile 0 leads with the prologue-sized groups
                groups = []
                g0 = 0
                if t == 0:
                    for j in range(NPRO):
                        groups.append((g0, PRO, pro_tiles[j]))
                        g0 += PRO
                while g0 < Kt:
                    groups.append((g0, min(GG, Kt - g0), None))
                    g0 += min(GG, Kt - g0)
                # stream the tile's chunks through small per-call buffers:
                # deep pool (bufs) keeps the Q7 descriptor gen far ahead
                for gi, (g0, gn, pre) in enumerate(groups):
                    if pre is not None:
                        gt = pre
                        # prepare_only consumer sync is author-managed:
                        # block DVE until this prologue DMA has landed
                        nc.vector.wait_ge(dma_sems[gi], 16)
                    else:
                        gt = ag.tile([P, GG, RW], BF16, tag="gt")
                        nc.gpsimd.dma_gather(
                            out_ap=gt[:, 0:gn, :],
                            in_ap=g[:],
                            idxs_ap=idxs_res[
                                :, off8 + g0 * 8 : off8 + (g0 + gn) * 8
                            ],
                            num_idxs=gn * P,
                            num_idxs_reg=gn * P,
                            elem_size=RW,
                            queue_num=ncall % NSWQ,
                        )
                        ncall += 1
                    # messages: ft *= ex (in-place; packed bf16 pairs)
                    nc.vector.tensor_tensor(
                        out=gt[:, 0:gn, :].rearrange(
                            "p k (q d2 w) -> p k q d2 w", q=BH, w=2
                        ),
                        in0=gt[:, 0:gn, :].rearrange(
                            "p k (q d2 w) -> p k q d2 w", q=BH, w=2
                        ),
                        in1=exv[:, g0 : g0 + gn, :, None, :].to_broadcast(
                            [P, gn, BH, D // 2, 2]
                        ),
                        op=OP.mult,
                    )
                    # segment sum via identity matmuls accumulating in PSUM
                    for k in range(gn):
                        nc.tensor.matmul(
                            acc[:],
                            lhsT=ident[:],
                            rhs=gt[:, k, :],
                            start=(g0 + k == 0),
                            stop=(g0 + k == Kt - 1),
                        )
                off8 += Kt * 8

                pending.append((t, acc, Kt))
                if len(pending) > 2:
                    _finalize(*pending.pop(0))
            for args in pending:
                _finalize(*args)

    nc.compile()
    return nc


def _make_inputs(feat, W, attn_l, attn_r, src, dst, n_nodes=N, n_cores=NCORES):
    feat = np.asarray(feat, dtype=np.float32)
    W = np.asarray(W, dtype=np.float32)
    attn_l = np.asarray(attn_l, dtype=np.float32)
    attn_r = np.asarray(attn_r, dtype=np.float32)
    src = np.asarray(src)
    dst = np.asarray(dst)

    K, idxs_all, sl_all, pos2node = _host_prep(src, dst)

    real = pos2node >= 0
    ftp = np.zeros((B, NPOS, DIN), np.float32)
    ftp[:, real, :] = feat[pos2node[real]].transpose(1, 0, 2)
    featT = np.ascontiguousarray(ftp.reshape(B * NPOS, DIN).T).astype(
        ml_dtypes.bfloat16
    )
    wmat = W.reshape(DIN, HD).astype(ml_dtypes.bfloat16)

    # host-folded attention logits (analogous to attn folding into W):
    # el/er per position, (NPOS+1, B*H) with the pad row's el = PAD_EL
    Wl = (W.reshape(DIN, H, D) * attn_l[None]).sum(-1)  # [DIN, H]
    Wr = (W.reshape(DIN, H, D) * attn_r[None]).sum(-1)
    el_pos = np.full((NPOS + 1, B, H), PAD_EL, np.float32)
    er_pos = np.zeros((NPOS + 1, B, H), np.float32)
    el_pos[:NPOS][real] = feat[pos2node[real]] @ Wl
    er_pos[:NPOS][real] = feat[pos2node[real]] @ Wr
    el_pos = el_pos.reshape(NPOS + 1, BH)
    er_pos = er_pos.reshape(NPOS + 1, BH)
    TCH = 5  # tiles per softmax chunk (matches _build)
    KC = [max(K[c0 : c0 + TCH]) for c0 in range(0, NT, TCH)]

    in_maps = []
    for c in range(n_cores):
        # slot-aligned el table padded per-chunk to Kc: [128, sum(5*Kc*BH)]
        parts = []
        off = 0
        for t in range(NT):
            Kt = K[t]
            Kc = KC[t // TCH]
            sl = sl_all[c][off : off + Kt * P]
            off += Kt * P
            ev = np.full((Kc, P, BH), PAD_EL, np.float32)
            ev[:Kt] = el_pos[sl].reshape(Kt, P, BH)
            parts.append(ev.transpose(1, 0, 2).reshape(P, Kc * BH))
        els = np.ascontiguousarray(np.concatenate(parts, axis=1)).astype(
            ml_dtypes.bfloat16
        )
        # own-position er table: [128, NT*BH]
        ero = np.ascontiguousarray(
            er_pos[c * NPOSC : (c + 1) * NPOSC]
            .reshape(NT, P, BH)
            .transpose(1, 0, 2)
            .reshape(P, NT * BH)
        ).astype(ml_dtypes.bfloat16)
        in_maps.append(
            {
                "featT": featT,
                "wmat": wmat,
                "idxs": idxs_all[c],
                "els": els,
                "ero": ero,
            }
        )
    return K, in_maps, pos2node


_CACHE = {}


def kernel(feat, W, attn_l, attn_r, src, dst):
    K, in_maps, pos2node = _make_inputs(feat, W, attn_l, attn_r, src, dst)
    key = tuple(K)
    if key not in _CACHE:
        _CACHE[key] = _build(K)
    nc = _CACHE[key]
    res = run_bass_kernel_spmd(nc, in_maps, list(range(NCORES))).results
    out = np.empty((N, B, H, D), np.float32)
    for c in range(NCORES):
        nodes = pos2node[c * NPOSC : c * NPOSC + NPC]
        out[nodes] = res[c]["out"][:NPC].astype(np.float32).reshape(NPC, B, H, D)
    return out


if __name__ == "__main__":
    rng = np.random.default_rng(0)
    feat = rng.standard_normal((N, B, DIN), dtype=np.float32)
    W = rng.standard_normal((DIN, H * D), dtype=np.float32) / np.sqrt(DIN)
    al = rng.standard_normal((H, D), dtype=np.float32) * 0.1
    ar = rng.standard_normal((H, D), dtype=np.float32) * 0.1
    src = rng.integers(0, N, E).astype(np.int32)
    dst = rng.integers(0, N, E).astype(np.int32)
    out = kernel(feat=feat, W=W, attn_l=al, attn_r=ar, src=src, dst=dst)
    print(out.shape, out.dtype, np.abs(out).mean())


# revision 23
# speedup vs baseline: 1.1944x; 1.1395x over previous
"""Trainium2 Bass kernel for BatchGATConv (GAT message passing).

Strategy (8 NeuronCores, SPMD, dst-partitioned):
  - Host: in-degree-sort each core's 2500 dst nodes into 20 tiles of 128 so
    every tile has near-uniform degree; chunk k of a tile holds the k-th
    in-edge of each of the tile's 128 nodes (slot-aligned, so dst-local ==
    partition and no one-hot matmuls are needed). Padded slots point at a
    dedicated zero pad row; their attention logit (host-supplied) is -150.
  - Attention logits el/er are folded on the host (el = feat @ (W*attn_l),
    an O(N*Din*H) matvec) and delivered slot-aligned, so gather rows carry
    only the projected features: 512 bf16 = 1024 B per edge, and the
    own-row gather chunk disappears.
  - All edge-softmax weights (exp(leaky(el+er)) in pair-duplicated layout,
    denominators, reciprocals) are computed up front from the host tables,
    overlapping the projection phase on DVE/ACT.
  - Projection (replicated on all cores, bf16): g[pos] = [ft(b0) | ft(b1)];
    two matmuls share one 2KB PSUM bank so PSUM->SBUF casts are [P,512].
  - Aggregation per tile: dma_gather fetches Kt*128 rows in <=1024-index
    calls rotated over 4 SWDGE queues; ft *= ex with a packed bf16
    innermost pair dim (DVE 2x mode); PSUM segment-sum via identity
    matmuls; out = Prelu(acc * rcp) in f16, cast to f32 on the host.
"""

import numpy as np

try:
    import concourse.bass as bass
except ImportError:  # pragma: no cover
    import sys

    sys.path.insert(0, "/opt/trn_rl_repo")
    import concourse.bass as bass

import ml_dtypes
import concourse.bacc as bacc
import concourse.mybir as mybir
import concourse.tile as tile
from concourse.bass_utils import run_bass_kernel_spmd

P = 128
F32 = mybir.dt.float32
F16 = mybir.dt.float16
BF16 = mybir.dt.bfloat16
I32 = mybir.dt.int32
I16 = mybir.dt.int16

# problem constants
N, B, DIN, H, D, E = 20000, 2, 128, 4, 64, 320000
NEG = 0.2
NCORES = 8
NPC = N // NCORES  # 2500 dst nodes per core
NT = 20  # node tiles per core
NPOSC = NT * P  # 2560 positions per core (60 pad positions)
NPOS = NCORES * NPOSC  # 20480 global positions
PADROW = NPOS  # pad row index (zeros)
GROWS = NPOS + 2
HD = H * D  # 256
BH = B * H  # 8
FT = B * HD  # 512
RW = 512  # g row width: [ft b0 | ft b1] bf16 = 1024 B (%256 for dma_gather)
PAD_EL = -150.0
QBLK = 2048  # projection rows per block
NBLK = NPOS // QBLK  # 10 per batch
NSWQ = 4  # SWDGE queues (Q7 descriptor-gen parallelism)


def _host_prep(src, dst):
    """Degree-sort nodes per core; build per-core wrapped int16 gather tables.

    Returns (K, idxs_all, sl_all, pos2node):
      K: per-tile chunk count, shared across cores (max in-degree in tile)
      idxs_all[c]: [128, sum(K)*8] int16 wrapped gather indices
      sl_all[c]: [sum(K)*P] int64 raw slot->position (PADROW = pad)
      pos2node: [NPOS] original node id per position (-1 = pad)
    """
    deg = np.bincount(dst, minlength=N).astype(np.int64)
    pos2node = np.full(NPOS, -1, np.int64)
    for c in range(NCORES):
        ids = np.arange(c * NPC, (c + 1) * NPC)
        order = np.argsort(-deg[ids], kind="stable")
        pos2node[c * NPOSC : c * NPOSC + NPC] = ids[order]
    node2pos = np.empty(N, np.int64)
    real = pos2node >= 0
    node2pos[pos2node[real]] = np.nonzero(real)[0]

    first = np.arange(NCORES)[:, None] * NPOSC + np.arange(NT)[None, :] * P
    K = np.maximum(deg[pos2node[first]].max(axis=0), 1)  # [NT]

    dpos = node2pos[dst]
    spos = node2pos[src]
    order_e = np.argsort(dpos, kind="stable")
    ds = dpos[order_e]
    ss = spos[order_e]
    starts = np.searchsorted(ds, np.arange(NPOS + 1))
    k_e = np.arange(E) - starts[ds]

    idxs_all = []
    sl_all = []
    for c in range(NCORES):
        cols = []
        slc = []
        for t in range(NT):
            Kt = int(K[t])
            base = c * NPOSC + t * P
            sl = np.full(Kt * P, PADROW, np.int64)
            e0, e1 = starts[base], starts[base + P]
            pp_ = ds[e0:e1] - base
            sl[k_e[e0:e1] * P + pp_] = ss[e0:e1]
            cols.append(sl.reshape(-1, 16).T)
            slc.append(sl)
        idxs_all.append(
            np.ascontiguousarray(
                np.tile(np.concatenate(cols, axis=1), (8, 1))
            ).astype(np.int16)
        )
        sl_all.append(np.concatenate(slc))
    return list(map(int, K)), idxs_all, sl_all, pos2node


def _build(K, stage='all'):
    S8 = sum(k * 8 for k in K)
    KM = max(K)
    TCH = 5  # tiles per softmax chunk
    KC = [max(K[c0 : c0 + TCH]) for c0 in range(0, NT, TCH)]
    SE = sum(TCH * kc * BH for kc in KC)  # per-chunk-padded el table
    AF = mybir.ActivationFunctionType
    OP = mybir.AluOpType

    nc = bacc.Bacc(trn_type="TRN2", num_swdge_queues=NSWQ)
    featT = nc.dram_tensor("featT", [DIN, B * NPOS], BF16, kind="ExternalInput")
    wmat = nc.dram_tensor("wmat", [DIN, HD], BF16, kind="ExternalInput")
    idxsd = nc.dram_tensor("idxs", [128, S8], I16, kind="ExternalInput")
    elsd = nc.dram_tensor("els", [128, SE], BF16, kind="ExternalInput")
    erod = nc.dram_tensor("ero", [128, NT * BH], BF16, kind="ExternalInput")
    outd = nc.dram_tensor("out", [NPOSC, FT], F16, kind="ExternalOutput")
    g = nc.dram_tensor("gtab", [GROWS, RW], BF16)

    with tile.TileContext(nc) as tc:
        with (
            tc.tile_pool(name="const", bufs=1) as cp,
            tc.tile_pool(name="proj", bufs=3) as pp,
            tc.tile_pool(name="projps", bufs=4, space="PSUM") as ppp,
            tc.tile_pool(name="agg", bufs=10) as ag,
            tc.tile_pool(name="small", bufs=3) as sm,
            tc.tile_pool(name="accps", bufs=3, space="PSUM") as psp,
        ):
            # resident constants
            w_sb = cp.tile([DIN, HD], BF16)
            nc.sync.dma_start(w_sb[:], wmat[:])
            idxs_res = cp.tile([128, S8], I16)
            nc.sync.dma_start(idxs_res[:], idxsd[:])
            el_sb = cp.tile([128, SE], BF16)
            nc.sync.dma_start(el_sb[:], elsd[:])
            er_sb = cp.tile([128, NT * BH], BF16)
            nc.sync.dma_start(er_sb[:], erod[:])
            iota_r = cp.tile([P, P], I32)
            nc.gpsimd.iota(iota_r[:], pattern=[[1, P]], base=0, channel_multiplier=0)
            iota_rf = cp.tile([P, P], F32)
            nc.vector.tensor_copy(iota_rf[:], iota_r[:])
            iota_p = cp.tile([P, 1], I32)
            nc.gpsimd.iota(iota_p[:], pattern=[[1, 1]], base=0, channel_multiplier=1)
            iota_pf = cp.tile([P, 1], F32)
            nc.vector.tensor_copy(iota_pf[:], iota_p[:])
            ident = cp.tile([P, P], BF16)
            nc.vector.tensor_scalar(
                out=ident[:],
                in0=iota_rf[:],
                scalar1=iota_pf[:, 0:1],
                scalar2=None,
                op0=OP.is_equal,
            )
            # pad row: ft = 0 (its logit lives in the host-built el table)
            padt = cp.tile([1, RW], BF16)
            nc.gpsimd.memset(padt[:], 0.0)
            nc.sync.dma_start(out=g[NPOS : NPOS + 1, :], in_=padt[:])

            # ---- per-tile softmax weights ----
            ex_tiles = []
            rcp_tiles = []
            eloff = 0
            for t in range(NT if stage in ('all', 'agg') else 0):
                Kt = K[t]
                Kc = KC[t // TCH]
                ext = cp.tile([P, Kc * BH * 2], BF16)
                exv = ext[:].rearrange("p (k q w) -> p k q w", q=BH, w=2)
                nc.vector.tensor_tensor(
                    out=exv[:, 0:Kc, :, :],
                    in0=el_sb[:, eloff : eloff + Kc * BH]
                    .rearrange("p (k q) -> p k q", q=BH)[:, :, :, None]
                    .to_broadcast([P, Kc, BH, 2]),
                    in1=er_sb[:, t * BH : (t + 1) * BH][:, None, :, None]
                    .to_broadcast([P, Kc, BH, 2]),
                    op=OP.add,
                )
                nc.scalar.activation(ext[:], ext[:], AF.Prelu, alpha=NEG)
                nc.scalar.activation(ext[:], ext[:], AF.Exp)
                dsum = sm.tile([P, 2 * BH], F32, tag="dsum")
                nc.vector.tensor_reduce(
                    out=dsum[:],
                    in_=exv.rearrange("p k q w -> p (q w) k"),
                    axis=mybir.AxisListType.X,
                    op=OP.add,
                )
                rct = cp.tile([P, 2 * BH], F32)
                nc.vector.reciprocal(rct[:], dsum[:])
                ex_tiles.append(ext[:])
                rcp_tiles.append(rct[:])
                eloff += Kc * BH
            # ---- projection: g[q] = [ft(b0) | ft(b1)] ----
            hw = (nc.sync, nc.scalar)
            for b in range(B if stage in ('all', 'proj') else 0):
                for blk in range(NBLK):
                    j = b * NBLK + blk
                    ftb = pp.tile([DIN, QBLK], BF16, tag="ftb")
                    hw[j % 2].dma_start(
                        ftb[:],
                        featT[:, b * NPOS + blk * QBLK : b * NPOS + (blk + 1) * QBLK],
                    )
                    pw = pp.tile([P, QBLK // P, HD], BF16, tag="pw")
                    for i in range(0, QBLK // P, 2):
                        po = ppp.tile([P, 2, HD], F32, tag="po")
                        for u in range(2):
                            nc.tensor.matmul(
                                po[:, u, :],
                                lhsT=ftb[:, (i + u) * P : (i + u + 1) * P],
                                rhs=w_sb[:],
                                start=True,
                                stop=True,
                            )
                        if i % 6 == 0:
                            nc.vector.tensor_copy(pw[:, i : i + 2, :], po[:])
                        else:
                            nc.scalar.activation(pw[:, i : i + 2, :], po[:], AF.Copy)
                    dst_ap = g[
                        blk * QBLK : (blk + 1) * QBLK, b * HD : (b + 1) * HD
                    ].rearrange("(i p) c -> p i c", p=P)
                    hw[(j + 1) % 2].dma_start(out=dst_ap, in_=pw[:])

            # ---- aggregation: per tile, slot-aligned weighted segment sum ----
            off8 = 0
            eloff = 0
            ncall = 0
            pending = []

            def _finalize(t, acc, Kt):
                o1 = sm.tile([P, FT], F32, tag="o1")
                nc.vector.tensor_tensor(
                    out=o1[:].rearrange("p (q d2 w) -> p q d2 w", q=BH, w=2),
                    in0=acc[:].rearrange("p (q d2 w) -> p q d2 w", q=BH, w=2),
                    in1=rcp_tiles[t]
                    .rearrange("p (q w) -> p q w", w=2)[:, :, None, :]
                    .to_broadcast([P, BH, D // 2, 2]),
                    op=OP.mult,
                )
                og = sm.tile([P, FT], F16, tag="og")
                nc.scalar.activation(og[:], o1[:], AF.Prelu, alpha=NEG)
                nc.sync.dma_start(out=outd[t * P : (t + 1) * P, :], in_=og[:])

            GG = 8  # chunks per gather group (== SWDGE ring cap of 1024 idx)
            for t in range(NT if stage in ('all', 'agg') else 0):
                Kt = K[t]
                acc = psp.tile([P, FT], F32, tag="acc")
                exv = ex_tiles[t].rearrange("p (k q w) -> p k q w", q=BH, w=2)
                # stream the tile's chunks through small per-call buffers:
                # deep pool (bufs) keeps the Q7 descriptor gen far ahead
                for g0 in range(0, Kt, GG):
                    gn = min(GG, Kt - g0)
                    gt = ag.tile([P, GG, RW], BF16, tag="gt")
                    nc.gpsimd.dma_gather(
                        out_ap=gt[:, 0:gn, :],
                        in_ap=g[:],
                        idxs_ap=idxs_res[:, off8 + g0 * 8 : off8 + (g0 + gn) * 8],
                        num_idxs=gn * P,
                        num_idxs_reg=gn * P,
                        elem_size=RW,
                        queue_num=ncall % NSWQ,
                    )
                    ncall += 1
                    # messages: ft *= ex (in-place; packed bf16 pairs)
                    nc.vector.tensor_tensor(
                        out=gt[:, 0:gn, :].rearrange(
                            "p k (q d2 w) -> p k q d2 w", q=BH, w=2
                        ),
                        in0=gt[:, 0:gn, :].rearrange(
                            "p k (q d2 w) -> p k q d2 w", q=BH, w=2
                        ),
                        in1=exv[:, g0 : g0 + gn, :, None, :].to_broadcast(
                            [P, gn, BH, D // 2, 2]
                        ),
                        op=OP.mult,
                    )
                    # segment sum via identity matmuls accumulating in PSUM
                    for k in range(gn):
                        nc.tensor.matmul(
                            acc[:],
                            lhsT=ident[:],
                            rhs=gt[:, k, :],
                            start=(g0 + k == 0),
                            stop=(g0 + k == Kt - 1),
                        )
                off8 += Kt * 8

                pending.append((t, acc, Kt))
                if len(pending) > 2:
                    _finalize(*pending.pop(0))
            for args in pending:
                _finalize(*args)

    nc.compile()
    return nc


def _make_inputs(feat, W, attn_l, attn_r, src, dst, n_nodes=N, n_cores=NCORES):
    feat = np.asarray(feat, dtype=np.float32)
    W = np.asarray(W, dtype=np.float32)
    attn_l = np.asarray(attn_l, dtype=np.float32)
    attn_r = np.asarray(attn_r, dtype=np.float32)
    src = np.asarray(src)
    dst = np.asarray(dst)

    K, idxs_all, sl_all, pos2node = _host_prep(src, dst)

    real = pos2node >= 0
    ftp = np.zeros((B, NPOS, DIN), np.float32)
    ftp[:, real, :] = feat[pos2node[real]].transpose(1, 0, 2)
    featT = np.ascontiguousarray(ftp.reshape(B * NPOS, DIN).T).astype(
        ml_dtypes.bfloat16
    )
    wmat = W.reshape(DIN, HD).astype(ml_dtypes.bfloat16)

    # host-folded attention logits (analogous to attn folding into W):
    # el/er per position, (NPOS+1, B*H) with the pad row's el = PAD_EL
    Wl = (W.reshape(DIN, H, D) * attn_l[None]).sum(-1)  # [DIN, H]
    Wr = (W.reshape(DIN, H, D) * attn_r[None]).sum(-1)
    el_pos = np.full((NPOS + 1, B, H), PAD_EL, np.float32)
    er_pos = np.zeros((NPOS + 1, B, H), np.float32)
    el_pos[:NPOS][real] = feat[pos2node[real]] @ Wl
    er_pos[:NPOS][real] = feat[pos2node[real]] @ Wr
    el_pos = el_pos.reshape(NPOS + 1, BH)
    er_pos = er_pos.reshape(NPOS + 1, BH)
    TCH = 5  # tiles per softmax chunk (matches _build)
    KC = [max(K[c0 : c0 + TCH]) for c0 in range(0, NT, TCH)]

    in_maps = []
    for c in range(n_cores):
        # slot-aligned el table padded per-chunk to Kc: [128, sum(5*Kc*BH)]
        parts = []
        off = 0
        for t in range(NT):
            Kt = K[t]
            Kc = KC[t // TCH]
            sl = sl_all[c][off : off + Kt * P]
            off += Kt * P
            ev = np.full((Kc, P, BH), PAD_EL, np.float32)
            ev[:Kt] = el_pos[sl].reshape(Kt, P, BH)
            parts.append(ev.transpose(1, 0, 2).reshape(P, Kc * BH))
        els = np.ascontiguousarray(np.concatenate(parts, axis=1)).astype(
            ml_dtypes.bfloat16
        )
        # own-position er table: [128, NT*BH]
        ero = np.ascontiguousarray(
            er_pos[c * NPOSC : (c + 1) * NPOSC]
            .reshape(NT, P, BH)
            .transpose(1, 0, 2)
            .reshape(P, NT * BH)
        ).astype(ml_dtypes.bfloat16)
        in_maps.append(
            {
                "featT": featT,
                "wmat": wmat,
                "idxs": idxs_all[c],
                "els": els,
                "ero": ero,
            }
        )
    return K, in_maps, pos2node


_CACHE = {}


def kernel(feat, W, attn_l, attn_r, src, dst):
    K, in_maps, pos2node = _make_inputs(feat, W, attn_l, attn_r, src, dst)
    key = tuple(K)
    if key not in _CACHE:
        _CACHE[key] = _build(K)
    nc = _CACHE[key]
    res = run_bass_kernel_spmd(nc, in_maps, list(range(NCORES))).results
    out = np.empty((N, B, H, D), np.float32)
    for c in range(NCORES):
        nodes = pos2node[c * NPOSC : c * NPOSC + NPC]
        out[nodes] = res[c]["out"][:NPC].astype(np.float32).reshape(NPC, B, H, D)
    return out


if __name__ == "__main__":
    rng = np.random.default_rng(0)
    feat = rng.standard_normal((N, B, DIN), dtype=np.float32)
    W = rng.standard_normal((DIN, H * D), dtype=np.float32) / np.sqrt(DIN)
    al = rng.standard_normal((H, D), dtype=np.float32) * 0.1
    ar = rng.standard_normal((H, D), dtype=np.float32) * 0.1
    src = rng.integers(0, N, E).astype(np.int32)
    dst = rng.integers(0, N, E).astype(np.int32)
    out = kernel(feat=feat, W=W, attn_l=al, attn_r=ar, src=src, dst=dst)
    print(out.shape, out.dtype, np.abs(out).mean())
